# revision 23
# baseline (speedup 1.0000x reference)
"""CenterFormer bbox head as a fused 3-stage matmul chain on 8 TRN2 cores.

Reference computation (per batch b, per proposal n):
  y = relu(BN(shared_w @ x + shared_b))            # 256 -> 64
  h = relu(BN(heads_w1[h] @ y + heads_b1[h]))      # 64 -> 64, 6 heads
  o = heads_w2[h] @ h + heads_b2[h]                # 64 -> 3 (padded), slice+concat -> 12

Host-side preprocessing folds BN (eval mode) into the conv weights, stacks the
6 head convs into a single [384, 64] matmul, and builds a block-diagonal
[12, 384] final conv that directly emits the channel-concatenated output.

Sharding: data-parallel over batch: core b handles ct_feat[b] ([256, 16384]).

Device kernel design (per core, N=16384 split into 32 tiles of F=512,
processed as 16 pairs):
  - bf16 matmuls (1 PE cycle/row, half the input DMA of fp32).
  - Every stationary is zero-padded to [128, 128] so every matmul runs with
    tile_size (128, 128): no PE array-mode switches, and stage-1/stage-3
    outputs pack two tiles into one PSUM bank (stage-1: y(jA) in partitions
    0-63, y(jB) in 64-127; stage-3: out(jA) in 0-11, out(jB) in 32-43).
  - PSUM budget: py x1 + ph x6 + po x1 = 8 banks.
  - The PE stream is software-pipelined: iteration p emits
    S1(p+1) | S2(p) | S3(p-2), so a matmul never waits on an eviction that
    was issued less than a full iteration (~3.4 us) earlier.
  - PSUM evictions (relu+bias / add-bias) are statically assigned to ACT
    and DVE only (GPSIMD cannot read PSUM): E1 ACT, E2 alternating DVE/ACT,
    E3 DVE -- 4 ops per engine per pair, both under the PE's 3.4 us.
"""

import numpy as np

BN_EPS = 1e-3
HEAD_CH = (3, 2, 1, 3, 2, 1)
B, CIN, N, CS, HN = 8, 256, 16384, 64, 6
COUT = sum(HEAD_CH)  # 12
NCORES = 8

MM_DTYPE = "bf16"

F = 512            # matmul free-dim tile (one fp32 PSUM bank)
PAIR = 2 * F       # two tiles processed per pipeline iteration
NPAIRS = N // PAIR  # 16

# packed stationary layout (columns of the [128, 608] weight tile):
#   cols 0-127:   stage-1: W1T k-chunks, [128, 64] each (shared by A/B halves
#                 via output column tiles)
#   cols 128-511: stage-2: W2T_m [128, 128], duplicated in both partition
#                 halves (row tiles)
#   cols 512-607: stage-3: W3T k-chunks padded to [128, 32] (shared by A/B
#                 via output column tiles)
W1_OFF, W2_OFF, W3_OFF, W_COLS = 0, 128, 512, 608
# bias tile [128, 5] f32: col0 [b1;b1], col1-3 b2 chunks, col4 b3 at rows
# {0-11, 32-43}
B1_COL, B2_COL, B3_COL, B_COLS = 0, 1, 4, 5

_CACHE: dict = {}


def _build_bass(mm_dtype: str, repeat: int = 1):
    import concourse.bacc as bacc
    import concourse.mybir as mybir
    from concourse.tile import TileContext

    f32 = mybir.dt.float32
    mdt = {"f32r": mybir.dt.float32r, "bf16": mybir.dt.bfloat16,
           "f16": mybir.dt.float16}[mm_dtype]
    AF = mybir.ActivationFunctionType

    nc = bacc.Bacc()
    x = nc.declare_dram_parameter("x", [CIN, N], mdt, isOutput=False)
    wp = nc.declare_dram_parameter("wp", [128, W_COLS], mdt, isOutput=False)
    bp = nc.declare_dram_parameter("bp", [128, B_COLS], f32, isOutput=False)
    out = nc.declare_dram_parameter("out", [COUT, N], mdt, isOutput=True)

    with TileContext(nc) as tc:
        with (
            tc.tile_pool(name="const", bufs=1) as cpool,
            tc.tile_pool(name="xin", bufs=4) as xpool,
            tc.tile_pool(name="acts", bufs=3) as apool,
            tc.tile_pool(name="outs", bufs=2) as opool,
            tc.tile_pool(name="psum", bufs=2, space="PSUM") as ppool,
        ):
            wt = cpool.tile([128, W_COLS], mdt)
            # stage-1 stationaries first so S1(0) can start early; the
            # stage-2/3 blocks (wpB) follow the first two x tiles so the
            # serialized DMA engines deliver S1's inputs first.
            nc.scalar.dma_start(out=wt[:, 0:W2_OFF], in_=wp[:, 0:W2_OFF])
            bt = cpool.tile([128, B_COLS], f32)
            nc.scalar.dma_start(out=bt[:], in_=bp[:])

            w1 = [wt[:, W1_OFF + k * 64 : W1_OFF + (k + 1) * 64]
                  for k in range(2)]
            w2 = [wt[:, W2_OFF + m * 128 : W2_OFF + (m + 1) * 128]
                  for m in range(3)]
            w3 = [wt[:, W3_OFF + k * 32 : W3_OFF + (k + 1) * 32]
                  for k in range(3)]
            b1 = bt[:, B1_COL : B1_COL + 1]
            b2 = [bt[:, B2_COL + m : B2_COL + m + 1] for m in range(3)]
            b3 = bt[0:44, B3_COL : B3_COL + 1]

            # Warm-ups: make PE/ACT observe the const DMAs via single-wait
            # ops so no later matmul needs a second sync-wait slot.
            pw = ppool.tile([1, 1], f32, tag="po", bufs=1)
            wwu = (wt[:, 0:1].bitcast(f32) if mm_dtype == "f32r"
                   else wt[:, 0:1])
            nc.tensor.matmul(pw[:], wwu, wwu, start=True, stop=True)
            sw = apool.tile([1, 1], f32, tag="warm", bufs=1)
            nc.scalar.activation(sw[:], bt[0:1, 0:1], AF.Copy)
            # stage-2/3 stationaries: issued after the warm-ups so the first
            # x tiles win the serialized DMA engines
            nc.scalar.dma_start(out=wt[:, W2_OFF:], in_=wp[:, W2_OFF:])

            xr = x.rearrange("(k p) n -> p k n", p=128)

            import contextlib
            loop_cm = (tc.For_i(0, repeat,
                                hint_engines=(mybir.EngineType.PE,))
                       if repeat > 1 else contextlib.nullcontext())

            def relu_bias(eng, dst, src, bias_ap):
                if eng == "ACT":
                    nc.scalar.activation(dst, src, AF.Relu, bias=bias_ap)
                elif eng == "DVE":
                    nc.vector.tensor_scalar(dst, src, bias_ap, 0.0,
                                            mybir.AluOpType.add,
                                            mybir.AluOpType.max)
                else:
                    nc.gpsimd.tensor_scalar(dst, src, bias_ap, 0.0,
                                            mybir.AluOpType.add,
                                            mybir.AluOpType.max)

            def add_bias(eng, dst, src, bias_ap):
                if eng == "ACT":
                    nc.scalar.activation(dst, src, AF.Identity, bias=bias_ap)
                elif eng == "DVE":
                    nc.vector.tensor_scalar(dst, src, bias_ap, None,
                                            mybir.AluOpType.add)
                else:
                    nc.gpsimd.tensor_scalar(dst, src, bias_ap, None,
                                            mybir.AluOpType.add)

            with loop_cm:
                xt = {}      # even pair -> x tile [128, 2, 2*PAIR] (2 pairs)
                ys = {}      # pair -> stage-1 output [128, F] (A|B packed)
                hs = {}      # pair -> list of 6 stage-2 outputs [128, F]
                ot2 = {}     # even pair -> [64, PAIR] out staging (2 pairs)

                def xdma(p):
                    xt[p] = xpool.tile([128, 2, PAIR], mdt, tag="xt",
                                       name=f"xt{p}")
                    nc.sync.dma_start(
                        out=xt[p][:],
                        in_=xr[:, :, p * PAIR : (p + 1) * PAIR])

                def s1(p):
                    # column tiles (128K, 64M): half A accumulates into py
                    # partitions 0-63, half B into 64-127; alternating tile
                    # positions (0,0)/(0,64) overlap on the PE
                    py = ppool.tile([128, F], f32, tag="py", bufs=1)
                    xa = xt[p][:, :, 0:F]
                    xb = xt[p][:, :, F:PAIR]
                    nc.tensor.matmul(py[0:64, :], w1[0], xa[:, 0],
                                     start=True, stop=False)
                    nc.tensor.matmul(py[64:128, :], w1[0], xb[:, 0],
                                     start=True, stop=False)
                    nc.tensor.matmul(py[0:64, :], w1[1], xa[:, 1],
                                     start=False, stop=True)
                    nc.tensor.matmul(py[64:128, :], w1[1], xb[:, 1],
                                     start=False, stop=True)
                    ys[p] = apool.tile([128, F], mdt, tag="ys", bufs=3, name=f"ys{p}")
                    relu_bias("ACT", ys[p][:], py[:], b1)
                    del xt[p]

                def s2(p):
                    # K=64 matmuls on alternating 64-row PE tiles (0,0)/(64,0)
                    # overlap on HW (~1.7x measured): W2T_m is duplicated in
                    # both partition halves of its block; half A contracts
                    # ys[0:64] (= y of tile jA), half B contracts ys[64:128]
                    hs[p] = []
                    engs = ("DVE", "ACT")
                    for i in range(6):
                        m, half = i // 2, i % 2
                        r0 = 64 * half
                        ph = ppool.tile([128, F], f32, tag="ph", bufs=6)
                        nc.tensor.matmul(ph[:],
                                         w2[m][r0 : r0 + 64, :],
                                         ys[p][r0 : r0 + 64, :],
                                         start=True, stop=True)
                        h = apool.tile([128, F], mdt, tag="hs", bufs=18, name=f"hs{p}_{i}")
                        relu_bias(engs[i % 2], h[:], ph[:], b2[m])
                        hs[p].append(h)
                    del ys[p]

                def s3(p):
                    # column tiles (128K, 32M): half A accumulates into pob
                    # partitions 0-31 (12 real + zero-pad), half B into
                    # 32-63; alternating positions (0,0)/(0,32) overlap
                    pob = ppool.tile([128, F], f32, tag="po", bufs=1)
                    for i in range(6):
                        k, half = i // 2, i % 2
                        c0 = 32 * half
                        nc.tensor.matmul(pob[c0 : c0 + 32, :], w3[k],
                                         hs[p][i][:],
                                         start=(i < 2), stop=(i >= 4))
                    ot = opool.tile([64, F], mdt, tag="ot")
                    add_bias("DVE", ot[0:44, :], pob[0:44, :], b3)
                    del hs[p]
                    c0 = p * PAIR
                    nc.sync.dma_start(out=out[:, c0 : c0 + F],
                                      in_=ot[0:COUT, :])
                    nc.sync.dma_start(out=out[:, c0 + F : c0 + PAIR],
                                      in_=ot[32 : 32 + COUT, :])

                # prologue: pair 0 arrives as two half-tiles so S1(0)'s
                # first matmuls start after ~700 ns of DMA instead of ~1.5 us
                xt[0] = xpool.tile([128, 2, PAIR], mdt, tag="xt", name="xt0")
                nc.sync.dma_start(out=xt[0][:, :, 0:F], in_=xr[:, :, 0:F])
                nc.sync.dma_start(out=xt[0][:, :, F:PAIR],
                                  in_=xr[:, :, F:PAIR])
                xdma(1)
                xdma(2)
                s1(0)

                for p in range(NPAIRS):
                    if p + 3 < NPAIRS:
                        xdma(p + 3)
                    if p + 1 < NPAIRS:
                        s1(p + 1)
                    s2(p)
                    if p >= 2:
                        s3(p - 2)
                s3(NPAIRS - 2)
                s3(NPAIRS - 1)

    nc.finalize()
    _check_matmul_waits(nc)
    return nc


def _check_matmul_waits(nc):
    import concourse.mybir as mybir

    bad = []
    for f in nc.m.functions:
        for blk in f.blocks:
            for inst in blk.instructions:
                if isinstance(inst, mybir.InstMatmult) and inst.sync_info:
                    if len(inst.sync_info.on_wait) > 1:
                        bad.append((inst.name,
                                    [w.ant_name for w in inst.sync_info.on_wait]))
    if bad:
        raise RuntimeError(f"matmuls with >1 sync wait (walrus limit): {bad}")


def _get_nc(mm_dtype: str, repeat: int = 1):
    key = (mm_dtype, repeat)
    if key not in _CACHE:
        _CACHE[key] = _build_bass(mm_dtype, repeat)
    return _CACHE[key]


def _np_mm_dtype(mm_dtype: str):
    if mm_dtype == "bf16":
        import ml_dtypes
        return ml_dtypes.bfloat16
    if mm_dtype == "f16":
        return np.float16
    return np.float32  # f32r streams fp32 bits


def _fold_params(inputs, mm_dtype: str):
    """Fold BN into conv weights; pack into the on-device tile layouts."""
    f = lambda k: np.asarray(inputs[k], np.float32)

    inv1 = f("shared_gamma") / np.sqrt(f("shared_var") + BN_EPS)          # [64]
    W1 = f("shared_w") * inv1[:, None]                                    # [64, 256]
    b1v = f("shared_b") * inv1 + f("shared_beta") - f("shared_mean") * inv1

    inv2 = f("heads_gamma") / np.sqrt(f("heads_var") + BN_EPS)            # [6, 64]
    W2 = (f("heads_w1") * inv2[:, :, None]).reshape(HN * CS, CS)          # [384, 64]
    b2v = (f("heads_b1") * inv2 + f("heads_beta")
           - f("heads_mean") * inv2).reshape(HN * CS)                     # [384]

    hw2, hb2 = f("heads_w2"), f("heads_b2")
    W3 = np.zeros((COUT, HN * CS), np.float32)                            # [12, 384]
    b3v = np.zeros((COUT,), np.float32)
    r = 0
    for h, ch in enumerate(HEAD_CH):
        W3[r : r + ch, h * CS : (h + 1) * CS] = hw2[h, :ch, :]
        b3v[r : r + ch] = hb2[h, :ch]
        r += ch

    # packed stationaries (see module docstring for the layout)
    wpk = np.zeros((128, W_COLS), np.float32)
    for k in range(2):                         # stage-1 k-chunks [128, 64]
        wpk[:, W1_OFF + k * 64 : W1_OFF + (k + 1) * 64] = \
            W1[:, k * 128 : (k + 1) * 128].T
    for m in range(3):                         # stage-2: W2T_m in both halves
        w2m = W2[m * 128 : (m + 1) * 128, :].T                            # [64, 128]
        wpk[0:64, W2_OFF + m * 128 : W2_OFF + (m + 1) * 128] = w2m
        wpk[64:128, W2_OFF + m * 128 : W2_OFF + (m + 1) * 128] = w2m
    for k in range(3):                         # stage-3 k-chunks [128, 32]
        wpk[:, W3_OFF + k * 32 : W3_OFF + k * 32 + COUT] = \
            W3[:, k * 128 : (k + 1) * 128].T

    bpk = np.zeros((128, B_COLS), np.float32)
    bpk[0:64, B1_COL] = b1v
    bpk[64:128, B1_COL] = b1v
    for m in range(3):
        bpk[:, B2_COL + m] = b2v[m * 128 : (m + 1) * 128]
    bpk[0:COUT, B3_COL] = b3v
    bpk[32 : 32 + COUT, B3_COL] = b3v

    wpk = wpk.astype(_np_mm_dtype(mm_dtype))
    return {"wp": wpk, "bp": bpk}


def _run(inputs, mm_dtype=MM_DTYPE, trace=False):
    from concourse.bass_utils import run_bass_kernel_spmd

    nc = _get_nc(mm_dtype)
    shared = _fold_params(inputs, mm_dtype)
    ct = np.asarray(inputs["ct_feat"], np.float32).astype(_np_mm_dtype(mm_dtype))
    in_maps = [
        {"x": np.ascontiguousarray(ct[b]), **shared} for b in range(B)
    ]
    res = run_bass_kernel_spmd(nc, in_maps, core_ids=list(range(NCORES)),
                               trace=trace)
    out = np.stack([np.asarray(res.results[b]["out"], np.float32)
                    for b in range(B)], axis=0)
    return out, res


def kernel(**inputs) -> np.ndarray:
    out, _ = _run(inputs)
    return out


# revision 25
# speedup vs baseline: 1.6365x; 1.6365x over previous
"""CenterFormer bbox head as a fused 3-stage matmul chain on 8 TRN2 cores.

Reference computation (per batch b, per proposal n):
  y = relu(BN(shared_w @ x + shared_b))            # 256 -> 64
  h = relu(BN(heads_w1[h] @ y + heads_b1[h]))      # 64 -> 64, 6 heads
  o = heads_w2[h] @ h + heads_b2[h]                # 64 -> 3 (padded), slice+concat -> 12

Host-side preprocessing folds BN (eval mode) into the conv weights, stacks the
6 head convs into a single [384, 64] matmul, and builds a block-diagonal
[12, 384] final conv that directly emits the channel-concatenated output.

Sharding: data-parallel over batch: core b handles ct_feat[b] ([256, 16384]).

Device kernel design (per core, N=16384 split into 32 tiles of F=512,
processed as 16 pairs):
  - bf16 end-to-end: x cast host-side (halves input DMA, the dominant cost),
    output written bf16 and upcast to f32 on the host.
  - PE-array tiling with measured instruction OVERLAP between disjoint tiles
    (~1.7x): stage 1 uses (128K, 64M) column tiles at positions (0,0)/(0,64)
    -- half A accumulates y(jA) into PSUM partitions 0-63, half B y(jB) into
    64-127; stage 2 uses (64K, 128M) row tiles at (0,0)/(64,0) with W2T_m
    duplicated in both partition halves; stage 3 uses (128K, 32M) column
    tiles at (0,0)/(0,32), so out(jA) lands in partitions 0-11 and out(jB)
    in 32-43 of one bank. Adjacent matmuls always alternate tile positions.
  - PSUM budget: py x1 + ph x6 + po x1 = 8 banks.
  - The PE stream is software-pipelined: iteration p emits
    S1(p+1) | S2(p) | S3(p-2), so a matmul never waits on an eviction that
    was issued less than a full iteration earlier.
  - PSUM evictions (relu+bias / add-bias) are statically assigned to ACT
    and DVE only (GPSIMD cannot read PSUM): E1 ACT, E2 alternating DVE/ACT,
    E3 DVE -- 4 ops per engine per pair.
  - All DMAs trigger from SP; input prefetch runs XT_BUFS pairs deep.
"""

import numpy as np

BN_EPS = 1e-3
HEAD_CH = (3, 2, 1, 3, 2, 1)
B, CIN, N, CS, HN = 8, 256, 16384, 64, 6
COUT = sum(HEAD_CH)  # 12
NCORES = 8

MM_DTYPE = "bf16"

F = 512            # matmul free-dim tile (one fp32 PSUM bank)
PAIR = 2 * F       # two tiles processed per pipeline iteration
NPAIRS = N // PAIR  # 16

# packed stationary layout (columns of the [128, 608] weight tile):
#   cols 0-127:   stage-1: W1T k-chunks, [128, 64] each (shared by A/B halves
#                 via output column tiles)
#   cols 128-511: stage-2: W2T_m [128, 128], duplicated in both partition
#                 halves (row tiles)
#   cols 512-607: stage-3: W3T k-chunks padded to [128, 32] (shared by A/B
#                 via output column tiles)
W1_OFF, W2_OFF, W3_OFF, W_COLS = 0, 128, 512, 608
# bias tile [128, 5] f32: col0 [b1;b1], col1-3 b2 chunks, col4 b3 at rows
# {0-11, 32-43}
B1_COL, B2_COL, B3_COL, B_COLS = 0, 1, 4, 5

_CACHE: dict = {}

# tuning knobs (part of the build cache key)
OUT_Q = "sp"      # "sp": both out-DMAs on SP; "split": one on ACT, one on Pool
XT_BUFS = 6       # input-tile prefetch depth


def _build_bass(mm_dtype: str, repeat: int = 1):
    import concourse.bacc as bacc
    import concourse.mybir as mybir
    from concourse.tile import TileContext

    f32 = mybir.dt.float32
    mdt = {"f32r": mybir.dt.float32r, "bf16": mybir.dt.bfloat16,
           "f16": mybir.dt.float16}[mm_dtype]
    AF = mybir.ActivationFunctionType

    nc = bacc.Bacc()
    x = nc.declare_dram_parameter("x", [CIN, N], mdt, isOutput=False)
    wp = nc.declare_dram_parameter("wp", [128, W_COLS], mdt, isOutput=False)
    bp = nc.declare_dram_parameter("bp", [128, B_COLS], f32, isOutput=False)
    out = nc.declare_dram_parameter("out", [COUT, N], mdt, isOutput=True)

    with TileContext(nc) as tc:
        with (
            tc.tile_pool(name="const", bufs=1) as cpool,
            tc.tile_pool(name="xin", bufs=XT_BUFS) as xpool,
            tc.tile_pool(name="acts", bufs=3) as apool,
            tc.tile_pool(name="outs", bufs=2) as opool,
            tc.tile_pool(name="psum", bufs=2, space="PSUM") as ppool,
        ):
            wt = cpool.tile([128, W_COLS], mdt)
            # stage-1 stationaries first so S1(0) can start early; the
            # stage-2/3 blocks (wpB) follow the first two x tiles so the
            # serialized DMA engines deliver S1's inputs first.
            nc.scalar.dma_start(out=wt[:, 0:W2_OFF], in_=wp[:, 0:W2_OFF])
            bt = cpool.tile([128, B_COLS], f32)
            nc.scalar.dma_start(out=bt[:], in_=bp[:])

            w1 = [wt[:, W1_OFF + k * 64 : W1_OFF + (k + 1) * 64]
                  for k in range(2)]
            w2 = [wt[:, W2_OFF + m * 128 : W2_OFF + (m + 1) * 128]
                  for m in range(3)]
            w3 = [wt[:, W3_OFF + k * 32 : W3_OFF + (k + 1) * 32]
                  for k in range(3)]
            b1 = bt[:, B1_COL : B1_COL + 1]
            b2 = [bt[:, B2_COL + m : B2_COL + m + 1] for m in range(3)]
            b3 = bt[0:44, B3_COL : B3_COL + 1]

            # Warm-ups: make PE/ACT observe the const DMAs via single-wait
            # ops so no later matmul needs a second sync-wait slot.
            pw = ppool.tile([1, 1], f32, tag="po", bufs=1)
            wwu = (wt[:, 0:1].bitcast(f32) if mm_dtype == "f32r"
                   else wt[:, 0:1])
            nc.tensor.matmul(pw[:], wwu, wwu, start=True, stop=True)
            sw = apool.tile([1, 1], f32, tag="warm", bufs=1)
            nc.scalar.activation(sw[:], bt[0:1, 0:1], AF.Copy)
            # stage-2/3 stationaries: issued after the warm-ups so the first
            # x tiles win the serialized DMA engines
            nc.scalar.dma_start(out=wt[:, W2_OFF:], in_=wp[:, W2_OFF:])

            xr = x.rearrange("(k p) n -> p k n", p=128)

            import contextlib
            loop_cm = (tc.For_i(0, repeat,
                                hint_engines=(mybir.EngineType.PE,))
                       if repeat > 1 else contextlib.nullcontext())

            def relu_bias(eng, dst, src, bias_ap):
                if eng == "ACT":
                    nc.scalar.activation(dst, src, AF.Relu, bias=bias_ap)
                elif eng == "DVE":
                    nc.vector.tensor_scalar(dst, src, bias_ap, 0.0,
                                            mybir.AluOpType.add,
                                            mybir.AluOpType.max)
                else:
                    nc.gpsimd.tensor_scalar(dst, src, bias_ap, 0.0,
                                            mybir.AluOpType.add,
                                            mybir.AluOpType.max)

            def add_bias(eng, dst, src, bias_ap):
                if eng == "ACT":
                    nc.scalar.activation(dst, src, AF.Identity, bias=bias_ap)
                elif eng == "DVE":
                    nc.vector.tensor_scalar(dst, src, bias_ap, None,
                                            mybir.AluOpType.add)
                else:
                    nc.gpsimd.tensor_scalar(dst, src, bias_ap, None,
                                            mybir.AluOpType.add)

            with loop_cm:
                xt = {}      # even pair -> x tile [128, 2, 2*PAIR] (2 pairs)
                ys = {}      # pair -> stage-1 output [128, F] (A|B packed)
                hs = {}      # pair -> list of 6 stage-2 outputs [128, F]
                ot2 = {}     # even pair -> [64, PAIR] out staging (2 pairs)

                def xdma(p):
                    xt[p] = xpool.tile([128, 2, PAIR], mdt, tag="xt",
                                       name=f"xt{p}")
                    nc.sync.dma_start(
                        out=xt[p][:],
                        in_=xr[:, :, p * PAIR : (p + 1) * PAIR])

                def s1(p):
                    # column tiles (128K, 64M): half A accumulates into py
                    # partitions 0-63, half B into 64-127; alternating tile
                    # positions (0,0)/(0,64) overlap on the PE
                    py = ppool.tile([128, F], f32, tag="py", bufs=1)
                    xa = xt[p][:, :, 0:F]
                    xb = xt[p][:, :, F:PAIR]
                    nc.tensor.matmul(py[0:64, :], w1[0], xa[:, 0],
                                     start=True, stop=False)
                    nc.tensor.matmul(py[64:128, :], w1[0], xb[:, 0],
                                     start=True, stop=False)
                    nc.tensor.matmul(py[0:64, :], w1[1], xa[:, 1],
                                     start=False, stop=True)
                    nc.tensor.matmul(py[64:128, :], w1[1], xb[:, 1],
                                     start=False, stop=True)
                    ys[p] = apool.tile([128, F], mdt, tag="ys", bufs=3, name=f"ys{p}")
                    relu_bias("ACT", ys[p][:], py[:], b1)
                    del xt[p]

                def s2(p):
                    # K=64 matmuls on alternating 64-row PE tiles (0,0)/(64,0)
                    # overlap on HW (~1.7x measured): W2T_m is duplicated in
                    # both partition halves of its block; half A contracts
                    # ys[0:64] (= y of tile jA), half B contracts ys[64:128]
                    hs[p] = []
                    engs = ("DVE", "ACT")
                    for i in range(6):
                        m, half = i // 2, i % 2
                        r0 = 64 * half
                        ph = ppool.tile([128, F], f32, tag="ph", bufs=6)
                        nc.tensor.matmul(ph[:],
                                         w2[m][r0 : r0 + 64, :],
                                         ys[p][r0 : r0 + 64, :],
                                         start=True, stop=True)
                        h = apool.tile([128, F], mdt, tag="hs", bufs=18, name=f"hs{p}_{i}")
                        relu_bias(engs[i % 2], h[:], ph[:], b2[m])
                        hs[p].append(h)
                    del ys[p]

                def s3(p):
                    # column tiles (128K, 32M): half A accumulates into pob
                    # partitions 0-31 (12 real + zero-pad), half B into
                    # 32-63; alternating positions (0,0)/(0,32) overlap
                    pob = ppool.tile([128, F], f32, tag="po", bufs=1)
                    for i in range(6):
                        k, half = i // 2, i % 2
                        c0 = 32 * half
                        nc.tensor.matmul(pob[c0 : c0 + 32, :], w3[k],
                                         hs[p][i][:],
                                         start=(i < 2), stop=(i >= 4))
                    ot = opool.tile([64, F], mdt, tag="ot")
                    add_bias("DVE", ot[0:44, :], pob[0:44, :], b3)
                    del hs[p]
                    c0 = p * PAIR
                    qa, qb = ((nc.sync, nc.sync) if OUT_Q == "sp"
                              else (nc.scalar, nc.gpsimd))
                    qa.dma_start(out=out[:, c0 : c0 + F],
                                 in_=ot[0:COUT, :])
                    qb.dma_start(out=out[:, c0 + F : c0 + PAIR],
                                 in_=ot[32 : 32 + COUT, :])

                # prologue: pair 0 arrives as two half-tiles so S1(0)'s
                # first matmuls start after ~700 ns of DMA instead of ~1.5 us
                xt[0] = xpool.tile([128, 2, PAIR], mdt, tag="xt", name="xt0")
                nc.sync.dma_start(out=xt[0][:, :, 0:F], in_=xr[:, :, 0:F])
                nc.sync.dma_start(out=xt[0][:, :, F:PAIR],
                                  in_=xr[:, :, F:PAIR])
                xdma(1)
                xdma(2)
                s1(0)

                for p in range(NPAIRS):
                    if p + 3 < NPAIRS:
                        xdma(p + 3)
                    if p + 1 < NPAIRS:
                        s1(p + 1)
                    s2(p)
                    if p >= 2:
                        s3(p - 2)
                s3(NPAIRS - 2)
                s3(NPAIRS - 1)

    nc.finalize()
    _check_matmul_waits(nc)
    return nc


def _check_matmul_waits(nc):
    import concourse.mybir as mybir

    bad = []
    for f in nc.m.functions:
        for blk in f.blocks:
            for inst in blk.instructions:
                if isinstance(inst, mybir.InstMatmult) and inst.sync_info:
                    if len(inst.sync_info.on_wait) > 1:
                        bad.append((inst.name,
                                    [w.ant_name for w in inst.sync_info.on_wait]))
    if bad:
        raise RuntimeError(f"matmuls with >1 sync wait (walrus limit): {bad}")


def _get_nc(mm_dtype: str, repeat: int = 1):
    key = (mm_dtype, repeat, OUT_Q, XT_BUFS)
    if key not in _CACHE:
        _CACHE[key] = _build_bass(mm_dtype, repeat)
    return _CACHE[key]


def _np_mm_dtype(mm_dtype: str):
    if mm_dtype == "bf16":
        import ml_dtypes
        return ml_dtypes.bfloat16
    if mm_dtype == "f16":
        return np.float16
    return np.float32  # f32r streams fp32 bits


def _fold_params(inputs, mm_dtype: str):
    """Fold BN into conv weights; pack into the on-device tile layouts."""
    f = lambda k: np.asarray(inputs[k], np.float32)

    inv1 = f("shared_gamma") / np.sqrt(f("shared_var") + BN_EPS)          # [64]
    W1 = f("shared_w") * inv1[:, None]                                    # [64, 256]
    b1v = f("shared_b") * inv1 + f("shared_beta") - f("shared_mean") * inv1

    inv2 = f("heads_gamma") / np.sqrt(f("heads_var") + BN_EPS)            # [6, 64]
    W2 = (f("heads_w1") * inv2[:, :, None]).reshape(HN * CS, CS)          # [384, 64]
    b2v = (f("heads_b1") * inv2 + f("heads_beta")
           - f("heads_mean") * inv2).reshape(HN * CS)                     # [384]

    hw2, hb2 = f("heads_w2"), f("heads_b2")
    W3 = np.zeros((COUT, HN * CS), np.float32)                            # [12, 384]
    b3v = np.zeros((COUT,), np.float32)
    r = 0
    for h, ch in enumerate(HEAD_CH):
        W3[r : r + ch, h * CS : (h + 1) * CS] = hw2[h, :ch, :]
        b3v[r : r + ch] = hb2[h, :ch]
        r += ch

    # packed stationaries (see module docstring for the layout)
    wpk = np.zeros((128, W_COLS), np.float32)
    for k in range(2):                         # stage-1 k-chunks [128, 64]
        wpk[:, W1_OFF + k * 64 : W1_OFF + (k + 1) * 64] = \
            W1[:, k * 128 : (k + 1) * 128].T
    for m in range(3):                         # stage-2: W2T_m in both halves
        w2m = W2[m * 128 : (m + 1) * 128, :].T                            # [64, 128]
        wpk[0:64, W2_OFF + m * 128 : W2_OFF + (m + 1) * 128] = w2m
        wpk[64:128, W2_OFF + m * 128 : W2_OFF + (m + 1) * 128] = w2m
    for k in range(3):                         # stage-3 k-chunks [128, 32]
        wpk[:, W3_OFF + k * 32 : W3_OFF + k * 32 + COUT] = \
            W3[:, k * 128 : (k + 1) * 128].T

    bpk = np.zeros((128, B_COLS), np.float32)
    bpk[0:64, B1_COL] = b1v
    bpk[64:128, B1_COL] = b1v
    for m in range(3):
        bpk[:, B2_COL + m] = b2v[m * 128 : (m + 1) * 128]
    bpk[0:COUT, B3_COL] = b3v
    bpk[32 : 32 + COUT, B3_COL] = b3v

    wpk = wpk.astype(_np_mm_dtype(mm_dtype))
    return {"wp": wpk, "bp": bpk}


def _run(inputs, mm_dtype=MM_DTYPE, trace=False):
    from concourse.bass_utils import run_bass_kernel_spmd

    nc = _get_nc(mm_dtype)
    shared = _fold_params(inputs, mm_dtype)
    ct = np.asarray(inputs["ct_feat"], np.float32).astype(_np_mm_dtype(mm_dtype))
    in_maps = [
        {"x": np.ascontiguousarray(ct[b]), **shared} for b in range(B)
    ]
    res = run_bass_kernel_spmd(nc, in_maps, core_ids=list(range(NCORES)),
                               trace=trace)
    out = np.stack([np.asarray(res.results[b]["out"], np.float32)
                    for b in range(B)], axis=0)
    return out, res


def kernel(**inputs) -> np.ndarray:
    out, _ = _run(inputs)
    return out


# revision 27
# speedup vs baseline: 2.0631x; 1.2607x over previous
"""CenterFormer bbox head as a fused 3-stage matmul chain on 8 TRN2 cores.

Reference computation (per batch b, per proposal n):
  y = relu(BN(shared_w @ x + shared_b))            # 256 -> 64
  h = relu(BN(heads_w1[h] @ y + heads_b1[h]))      # 64 -> 64, 6 heads
  o = heads_w2[h] @ h + heads_b2[h]                # 64 -> 3 (padded), slice+concat -> 12

Host-side preprocessing folds BN (eval mode) into the conv weights, stacks the
6 head convs into a single [384, 64] matmul, and builds a block-diagonal
[12, 384] final conv that directly emits the channel-concatenated output.

Sharding: data-parallel over batch: core b handles ct_feat[b] ([256, 16384]).

Device kernel design (per core, N=16384 split into 32 tiles of F=512,
processed as 16 pairs):
  - bf16 end-to-end: x cast host-side (halves input DMA, the dominant cost),
    output written bf16 and upcast to f32 on the host.
  - PE-array tiling with measured instruction OVERLAP between disjoint tiles
    (~1.7x): stage 1 uses (128K, 64M) column tiles at positions (0,0)/(0,64)
    -- half A accumulates y(jA) into PSUM partitions 0-63, half B y(jB) into
    64-127; stage 2 uses (64K, 128M) row tiles at (0,0)/(64,0) with W2T_m
    duplicated in both partition halves; stage 3 uses (128K, 32M) column
    tiles at (0,0)/(0,32), so out(jA) lands in partitions 0-11 and out(jB)
    in 32-43 of one bank. Adjacent matmuls always alternate tile positions.
  - PSUM budget: py x1 + ph x6 + po x1 = 8 banks.
  - The PE stream is software-pipelined: iteration p emits
    S1(p+1) | S2(p) | S3(p-2), so a matmul never waits on an eviction that
    was issued less than a full iteration earlier.
  - PSUM evictions (relu+bias / add-bias) are statically assigned to ACT
    and DVE only (GPSIMD cannot read PSUM): E1 ACT, E2 alternating DVE/ACT,
    E3 DVE -- 4 ops per engine per pair.
  - All DMAs trigger from SP; input prefetch runs XT_BUFS pairs deep.
"""

import numpy as np

BN_EPS = 1e-3
HEAD_CH = (3, 2, 1, 3, 2, 1)
B, CIN, N, CS, HN = 8, 256, 16384, 64, 6
COUT = sum(HEAD_CH)  # 12
NCORES = 8

MM_DTYPE = "bf16"

F = 512            # matmul free-dim tile (one fp32 PSUM bank)
PAIR = 2 * F       # two tiles processed per pipeline iteration
NPAIRS = N // PAIR  # 16

# packed stationary layout (columns of the [128, 608] weight tile):
#   cols 0-127:   stage-1: W1T k-chunks, [128, 64] each (shared by A/B halves
#                 via output column tiles)
#   cols 128-511: stage-2: W2T_m [128, 128], duplicated in both partition
#                 halves (row tiles)
#   cols 512-607: stage-3: W3T k-chunks padded to [128, 32] (shared by A/B
#                 via output column tiles)
W1_OFF, W2_OFF, W3_OFF, W_COLS = 0, 128, 512, 608
# bias tile [128, 5] f32: col0 [b1;b1], col1-3 b2 chunks, col4 b3 at rows
# {0-11, 32-43}
B1_COL, B2_COL, B3_COL, B_COLS = 0, 1, 4, 5

_CACHE: dict = {}

# tuning knobs (part of the build cache key)
OUT_Q = "sp"      # "sp": both out-DMAs on SP; "split": one on ACT, one on Pool
XT_BUFS = 6       # input-tile prefetch depth


def _build_bass(mm_dtype: str, repeat: int = 1):
    import concourse.bacc as bacc
    import concourse.mybir as mybir
    from concourse.tile import TileContext

    f32 = mybir.dt.float32
    mdt = {"f32r": mybir.dt.float32r, "bf16": mybir.dt.bfloat16,
           "f16": mybir.dt.float16}[mm_dtype]
    AF = mybir.ActivationFunctionType

    nc = bacc.Bacc()
    x = nc.declare_dram_parameter("x", [CIN, N], mdt, isOutput=False)
    wp = nc.declare_dram_parameter("wp", [128, W_COLS], mdt, isOutput=False)
    bp = nc.declare_dram_parameter("bp", [128, B_COLS], f32, isOutput=False)
    out = nc.declare_dram_parameter("out", [COUT, N], mdt, isOutput=True)

    with TileContext(nc) as tc:
        with (
            tc.tile_pool(name="const", bufs=1) as cpool,
            tc.tile_pool(name="xin", bufs=XT_BUFS) as xpool,
            tc.tile_pool(name="acts", bufs=3) as apool,
            tc.tile_pool(name="outs", bufs=2) as opool,
            tc.tile_pool(name="psum", bufs=2, space="PSUM") as ppool,
        ):
            wt = cpool.tile([128, W_COLS], mdt)
            # stage-1 stationaries first so S1(0) can start early; the
            # stage-2/3 blocks (wpB) follow the first two x tiles so the
            # serialized DMA engines deliver S1's inputs first.
            nc.scalar.dma_start(out=wt[:, 0:W2_OFF], in_=wp[:, 0:W2_OFF])
            bt = cpool.tile([128, B_COLS], f32)
            nc.scalar.dma_start(out=bt[:], in_=bp[:])

            w1 = [wt[:, W1_OFF + k * 64 : W1_OFF + (k + 1) * 64]
                  for k in range(2)]
            w2 = [wt[:, W2_OFF + m * 128 : W2_OFF + (m + 1) * 128]
                  for m in range(3)]
            w3 = [wt[:, W3_OFF + k * 32 : W3_OFF + (k + 1) * 32]
                  for k in range(3)]
            b1 = bt[:, B1_COL : B1_COL + 1]
            b2 = [bt[:, B2_COL + m : B2_COL + m + 1] for m in range(3)]
            b3 = bt[0:44, B3_COL : B3_COL + 1]

            # Warm-ups: make PE/ACT observe the const DMAs via single-wait
            # ops so no later matmul needs a second sync-wait slot.
            pw = ppool.tile([1, 1], f32, tag="po", bufs=1)
            wwu = (wt[:, 0:1].bitcast(f32) if mm_dtype == "f32r"
                   else wt[:, 0:1])
            nc.tensor.matmul(pw[:], wwu, wwu, start=True, stop=True)
            sw = apool.tile([1, 1], f32, tag="warm", bufs=1)
            nc.scalar.activation(sw[:], bt[0:1, 0:1], AF.Copy)
            # stage-2/3 stationaries: issued after the warm-ups so the first
            # x tiles win the serialized DMA engines
            nc.scalar.dma_start(out=wt[:, W2_OFF:], in_=wp[:, W2_OFF:])

            xr = x.rearrange("(k p) n -> p k n", p=128)

            import contextlib
            loop_cm = (tc.For_i(0, repeat,
                                hint_engines=(mybir.EngineType.PE,))
                       if repeat > 1 else contextlib.nullcontext())

            def relu_bias(eng, dst, src, bias_ap):
                if eng == "ACT":
                    nc.scalar.activation(dst, src, AF.Relu, bias=bias_ap)
                elif eng == "DVE":
                    nc.vector.tensor_scalar(dst, src, bias_ap, 0.0,
                                            mybir.AluOpType.add,
                                            mybir.AluOpType.max)
                else:
                    nc.gpsimd.tensor_scalar(dst, src, bias_ap, 0.0,
                                            mybir.AluOpType.add,
                                            mybir.AluOpType.max)

            def add_bias(eng, dst, src, bias_ap):
                if eng == "ACT":
                    nc.scalar.activation(dst, src, AF.Identity, bias=bias_ap)
                elif eng == "DVE":
                    nc.vector.tensor_scalar(dst, src, bias_ap, None,
                                            mybir.AluOpType.add)
                else:
                    nc.gpsimd.tensor_scalar(dst, src, bias_ap, None,
                                            mybir.AluOpType.add)

            with loop_cm:
                xt = {}      # even pair -> x tile [128, 2, 2*PAIR] (2 pairs)
                ys = {}      # pair -> stage-1 output [128, F] (A|B packed)
                hs = {}      # pair -> list of 6 stage-2 outputs [128, F]
                ot2 = {}     # even pair -> [64, PAIR] out staging (2 pairs)

                def xdma(p):
                    xt[p] = xpool.tile([128, 2, PAIR], mdt, tag="xt",
                                       name=f"xt{p}")
                    nc.sync.dma_start(
                        out=xt[p][:],
                        in_=xr[:, :, p * PAIR : (p + 1) * PAIR])

                def s1(p):
                    # column tiles (128K, 64M): half A accumulates into py
                    # partitions 0-63, half B into 64-127; alternating tile
                    # positions (0,0)/(0,64) overlap on the PE
                    py = ppool.tile([128, F], f32, tag="py", bufs=1)
                    xa = xt[p][:, :, 0:F]
                    xb = xt[p][:, :, F:PAIR]
                    nc.tensor.matmul(py[0:64, :], w1[0], xa[:, 0],
                                     start=True, stop=False)
                    nc.tensor.matmul(py[64:128, :], w1[0], xb[:, 0],
                                     start=True, stop=False)
                    nc.tensor.matmul(py[0:64, :], w1[1], xa[:, 1],
                                     start=False, stop=True)
                    nc.tensor.matmul(py[64:128, :], w1[1], xb[:, 1],
                                     start=False, stop=True)
                    ys[p] = apool.tile([128, F], mdt, tag="ys", bufs=3, name=f"ys{p}")
                    relu_bias("DVE", ys[p][:], py[:], b1)
                    del xt[p]

                def s2(p):
                    # K=64 matmuls on alternating 64-row PE tiles (0,0)/(64,0)
                    # overlap on HW (~1.7x measured): W2T_m is duplicated in
                    # both partition halves of its block; half A contracts
                    # ys[0:64] (= y of tile jA), half B contracts ys[64:128].
                    # Each m's A/B results land in one [128, 2, F] double-bank
                    # psum tile and evict with a single [128, 2F] op.
                    hs[p] = []
                    engs = ("ACT", "DVE", "ACT")
                    for m in range(3):
                        phd = ppool.tile([128, 2, F], f32, tag="ph", bufs=3,
                                         name=f"ph{p}_{m}")
                        for half in range(2):
                            r0 = 64 * half
                            nc.tensor.matmul(phd[:, half, :],
                                             w2[m][r0 : r0 + 64, :],
                                             ys[p][r0 : r0 + 64, :],
                                             start=True, stop=True)
                        hd = apool.tile([128, 2, F], mdt, tag="hs", bufs=9,
                                        name=f"hs{p}_{m}")
                        relu_bias(engs[m], hd[:], phd[:], b2[m])
                        hs[p].extend([hd[:, 0, :], hd[:, 1, :]])
                    del ys[p]

                def s3(p):
                    # column tiles (128K, 32M): half A accumulates into pob
                    # partitions 0-31 (12 real + zero-pad), half B into
                    # 32-63; alternating positions (0,0)/(0,32) overlap
                    pob = ppool.tile([128, F], f32, tag="po", bufs=1)
                    for i in range(6):
                        k, half = i // 2, i % 2
                        c0 = 32 * half
                        nc.tensor.matmul(pob[c0 : c0 + 32, :], w3[k],
                                         hs[p][i],
                                         start=(i < 2), stop=(i >= 4))
                    ot = opool.tile([64, F], mdt, tag="ot")
                    add_bias("DVE", ot[0:44, :], pob[0:44, :], b3)
                    del hs[p]
                    c0 = p * PAIR
                    qa, qb = {"sp": (nc.sync, nc.sync),
                              "split": (nc.scalar, nc.gpsimd),
                              "pool2": (nc.gpsimd, nc.gpsimd)}[OUT_Q]
                    qa.dma_start(out=out[:, c0 : c0 + F],
                                 in_=ot[0:COUT, :])
                    qb.dma_start(out=out[:, c0 + F : c0 + PAIR],
                                 in_=ot[32 : 32 + COUT, :])

                # prologue: pair 0 arrives as two half-tiles so S1(0)'s
                # first matmuls start after ~700 ns of DMA instead of ~1.5 us
                xt[0] = xpool.tile([128, 2, PAIR], mdt, tag="xt", name="xt0")
                nc.sync.dma_start(out=xt[0][:, :, 0:F], in_=xr[:, :, 0:F])
                nc.sync.dma_start(out=xt[0][:, :, F:PAIR],
                                  in_=xr[:, :, F:PAIR])
                xdma(1)
                xdma(2)
                s1(0)

                for p in range(NPAIRS):
                    if p + 3 < NPAIRS:
                        xdma(p + 3)
                    if p + 1 < NPAIRS:
                        s1(p + 1)
                    s2(p)
                    if p >= 2:
                        s3(p - 2)
                s3(NPAIRS - 2)
                s3(NPAIRS - 1)

    nc.finalize()
    _check_matmul_waits(nc)
    return nc


def _check_matmul_waits(nc):
    import concourse.mybir as mybir

    bad = []
    for f in nc.m.functions:
        for blk in f.blocks:
            for inst in blk.instructions:
                if isinstance(inst, mybir.InstMatmult) and inst.sync_info:
                    if len(inst.sync_info.on_wait) > 1:
                        bad.append((inst.name,
                                    [w.ant_name for w in inst.sync_info.on_wait]))
    if bad:
        raise RuntimeError(f"matmuls with >1 sync wait (walrus limit): {bad}")


def _get_nc(mm_dtype: str, repeat: int = 1):
    key = (mm_dtype, repeat, OUT_Q, XT_BUFS)
    if key not in _CACHE:
        _CACHE[key] = _build_bass(mm_dtype, repeat)
    return _CACHE[key]


def _np_mm_dtype(mm_dtype: str):
    if mm_dtype == "bf16":
        import ml_dtypes
        return ml_dtypes.bfloat16
    if mm_dtype == "f16":
        return np.float16
    return np.float32  # f32r streams fp32 bits


def _fold_params(inputs, mm_dtype: str):
    """Fold BN into conv weights; pack into the on-device tile layouts."""
    f = lambda k: np.asarray(inputs[k], np.float32)

    inv1 = f("shared_gamma") / np.sqrt(f("shared_var") + BN_EPS)          # [64]
    W1 = f("shared_w") * inv1[:, None]                                    # [64, 256]
    b1v = f("shared_b") * inv1 + f("shared_beta") - f("shared_mean") * inv1

    inv2 = f("heads_gamma") / np.sqrt(f("heads_var") + BN_EPS)            # [6, 64]
    W2 = (f("heads_w1") * inv2[:, :, None]).reshape(HN * CS, CS)          # [384, 64]
    b2v = (f("heads_b1") * inv2 + f("heads_beta")
           - f("heads_mean") * inv2).reshape(HN * CS)                     # [384]

    hw2, hb2 = f("heads_w2"), f("heads_b2")
    W3 = np.zeros((COUT, HN * CS), np.float32)                            # [12, 384]
    b3v = np.zeros((COUT,), np.float32)
    r = 0
    for h, ch in enumerate(HEAD_CH):
        W3[r : r + ch, h * CS : (h + 1) * CS] = hw2[h, :ch, :]
        b3v[r : r + ch] = hb2[h, :ch]
        r += ch

    # packed stationaries (see module docstring for the layout)
    wpk = np.zeros((128, W_COLS), np.float32)
    for k in range(2):                         # stage-1 k-chunks [128, 64]
        wpk[:, W1_OFF + k * 64 : W1_OFF + (k + 1) * 64] = \
            W1[:, k * 128 : (k + 1) * 128].T
    for m in range(3):                         # stage-2: W2T_m in both halves
        w2m = W2[m * 128 : (m + 1) * 128, :].T                            # [64, 128]
        wpk[0:64, W2_OFF + m * 128 : W2_OFF + (m + 1) * 128] = w2m
        wpk[64:128, W2_OFF + m * 128 : W2_OFF + (m + 1) * 128] = w2m
    for k in range(3):                         # stage-3 k-chunks [128, 32]
        wpk[:, W3_OFF + k * 32 : W3_OFF + k * 32 + COUT] = \
            W3[:, k * 128 : (k + 1) * 128].T

    bpk = np.zeros((128, B_COLS), np.float32)
    bpk[0:64, B1_COL] = b1v
    bpk[64:128, B1_COL] = b1v
    for m in range(3):
        bpk[:, B2_COL + m] = b2v[m * 128 : (m + 1) * 128]
    bpk[0:COUT, B3_COL] = b3v
    bpk[32 : 32 + COUT, B3_COL] = b3v

    wpk = wpk.astype(_np_mm_dtype(mm_dtype))
    return {"wp": wpk, "bp": bpk}


def _run(inputs, mm_dtype=MM_DTYPE, trace=False):
    from concourse.bass_utils import run_bass_kernel_spmd

    nc = _get_nc(mm_dtype)
    shared = _fold_params(inputs, mm_dtype)
    ct = np.asarray(inputs["ct_feat"], np.float32).astype(_np_mm_dtype(mm_dtype))
    in_maps = [
        {"x": np.ascontiguousarray(ct[b]), **shared} for b in range(B)
    ]
    res = run_bass_kernel_spmd(nc, in_maps, core_ids=list(range(NCORES)),
                               trace=trace)
    out = np.stack([np.asarray(res.results[b]["out"], np.float32)
                    for b in range(B)], axis=0)
    return out, res


def kernel(**inputs) -> np.ndarray:
    out, _ = _run(inputs)
    return out


# revision 28
# speedup vs baseline: 2.4829x; 1.2035x over previous
"""CenterFormer bbox head as a fused 3-stage matmul chain on 8 TRN2 cores.

Reference computation (per batch b, per proposal n):
  y = relu(BN(shared_w @ x + shared_b))            # 256 -> 64
  h = relu(BN(heads_w1[h] @ y + heads_b1[h]))      # 64 -> 64, 6 heads
  o = heads_w2[h] @ h + heads_b2[h]                # 64 -> 3 (padded), slice+concat -> 12

Host-side preprocessing folds BN (eval mode) into the conv weights, stacks the
6 head convs into a single [384, 64] matmul, and builds a block-diagonal
[12, 384] final conv that directly emits the channel-concatenated output.

Sharding: data-parallel over batch: core b handles ct_feat[b] ([256, 16384]).

Device kernel design (per core, N=16384 split into 32 tiles of F=512,
processed as 16 pairs):
  - bf16 end-to-end: x cast host-side (halves input DMA, the dominant cost),
    output written bf16 and upcast to f32 on the host.
  - PE-array tiling with measured instruction OVERLAP between disjoint tiles
    (~1.7x): stage 1 uses (128K, 64M) column tiles at positions (0,0)/(0,64)
    -- half A accumulates y(jA) into PSUM partitions 0-63, half B y(jB) into
    64-127; stage 2 uses (64K, 128M) row tiles at (0,0)/(64,0) with W2T_m
    duplicated in both partition halves; stage 3 uses (128K, 32M) column
    tiles at (0,0)/(0,32), so out(jA) lands in partitions 0-11 and out(jB)
    in 32-43 of one bank. Adjacent matmuls always alternate tile positions.
  - PSUM budget: py x1 + ph x6 + po x1 = 8 banks.
  - The PE stream is software-pipelined: iteration p emits
    S1(p+1) | S2(p) | S3(p-2), so a matmul never waits on an eviction that
    was issued less than a full iteration earlier.
  - PSUM evictions (relu+bias / add-bias) are statically assigned to ACT
    and DVE only (GPSIMD cannot read PSUM): E1 ACT, E2 alternating DVE/ACT,
    E3 DVE -- 4 ops per engine per pair.
  - All DMAs trigger from SP; input prefetch runs XT_BUFS pairs deep.
"""

import numpy as np

BN_EPS = 1e-3
HEAD_CH = (3, 2, 1, 3, 2, 1)
B, CIN, N, CS, HN = 8, 256, 16384, 64, 6
COUT = sum(HEAD_CH)  # 12
NCORES = 8

MM_DTYPE = "bf16"

F = 512            # matmul free-dim tile (one fp32 PSUM bank)
PAIR = 2 * F       # two tiles processed per pipeline iteration
NPAIRS = N // PAIR  # 16

# packed stationary layout (columns of the [128, 608] weight tile):
#   cols 0-127:   stage-1: W1T k-chunks, [128, 64] each (shared by A/B halves
#                 via output column tiles)
#   cols 128-511: stage-2: W2T_m [128, 128], duplicated in both partition
#                 halves (row tiles)
#   cols 512-607: stage-3: W3T k-chunks padded to [128, 32] (shared by A/B
#                 via output column tiles)
W1_OFF, W2_OFF, W3_OFF, W_COLS = 0, 128, 512, 608
# bias tile [128, 5] f32: col0 [b1;b1], col1-3 b2 chunks, col4 b3 at rows
# {0-11, 32-43}
B1_COL, B2_COL, B3_COL, B_COLS = 0, 1, 4, 5

_CACHE: dict = {}

# tuning knobs (part of the build cache key)
OUT_Q = "sp"      # "sp": both out-DMAs on SP; "split": one on ACT, one on Pool
XT_BUFS = 6       # input-tile prefetch depth


def _build_bass(mm_dtype: str, repeat: int = 1):
    import concourse.bacc as bacc
    import concourse.mybir as mybir
    from concourse.tile import TileContext

    f32 = mybir.dt.float32
    mdt = {"f32r": mybir.dt.float32r, "bf16": mybir.dt.bfloat16,
           "f16": mybir.dt.float16}[mm_dtype]
    AF = mybir.ActivationFunctionType

    nc = bacc.Bacc()
    x = nc.declare_dram_parameter("x", [CIN, N], mdt, isOutput=False)
    wp = nc.declare_dram_parameter("wp", [128, W_COLS], mdt, isOutput=False)
    bp = nc.declare_dram_parameter("bp", [128, B_COLS], f32, isOutput=False)
    out = nc.declare_dram_parameter("out", [COUT, N], mdt, isOutput=True)

    with TileContext(nc) as tc:
        with (
            tc.tile_pool(name="const", bufs=1) as cpool,
            tc.tile_pool(name="xin", bufs=XT_BUFS) as xpool,
            tc.tile_pool(name="acts", bufs=3) as apool,
            tc.tile_pool(name="outs", bufs=2) as opool,
            tc.tile_pool(name="psum", bufs=2, space="PSUM") as ppool,
        ):
            wt = cpool.tile([128, W_COLS], mdt)
            # stage-1 stationaries first so S1(0) can start early; the
            # stage-2/3 blocks (wpB) follow the first two x tiles so the
            # serialized DMA engines deliver S1's inputs first.
            nc.scalar.dma_start(out=wt[:, 0:W2_OFF], in_=wp[:, 0:W2_OFF])
            bt = cpool.tile([128, B_COLS], f32)
            nc.scalar.dma_start(out=bt[:], in_=bp[:])

            w1 = [wt[:, W1_OFF + k * 64 : W1_OFF + (k + 1) * 64]
                  for k in range(2)]
            w2 = [wt[:, W2_OFF + m * 128 : W2_OFF + (m + 1) * 128]
                  for m in range(3)]
            w3 = [wt[:, W3_OFF + k * 32 : W3_OFF + (k + 1) * 32]
                  for k in range(3)]
            b1 = bt[:, B1_COL : B1_COL + 1]
            b2 = [bt[:, B2_COL + m : B2_COL + m + 1] for m in range(3)]
            b3 = bt[0:44, B3_COL : B3_COL + 1]

            # stage-2/3 stationaries issued separately so the first x tiles
            # win the serialized DMA engines. No warm-up ops: bf16 matmuls
            # lower to Ldweights+Matmult, so bacc can split a matmul's two
            # DMA waits across the pair (_check_matmul_waits verifies).
            nc.scalar.dma_start(out=wt[:, W2_OFF:], in_=wp[:, W2_OFF:])

            xr = x.rearrange("(k p) n -> p k n", p=128)

            import contextlib
            loop_cm = (tc.For_i(0, repeat,
                                hint_engines=(mybir.EngineType.PE,))
                       if repeat > 1 else contextlib.nullcontext())

            def relu_bias(eng, dst, src, bias_ap):
                if eng == "ACT":
                    nc.scalar.activation(dst, src, AF.Relu, bias=bias_ap)
                elif eng == "DVE":
                    nc.vector.tensor_scalar(dst, src, bias_ap, 0.0,
                                            mybir.AluOpType.add,
                                            mybir.AluOpType.max)
                else:
                    nc.gpsimd.tensor_scalar(dst, src, bias_ap, 0.0,
                                            mybir.AluOpType.add,
                                            mybir.AluOpType.max)

            def add_bias(eng, dst, src, bias_ap):
                if eng == "ACT":
                    nc.scalar.activation(dst, src, AF.Identity, bias=bias_ap)
                elif eng == "DVE":
                    nc.vector.tensor_scalar(dst, src, bias_ap, None,
                                            mybir.AluOpType.add)
                else:
                    nc.gpsimd.tensor_scalar(dst, src, bias_ap, None,
                                            mybir.AluOpType.add)

            with loop_cm:
                xt = {}      # even pair -> x tile [128, 2, 2*PAIR] (2 pairs)
                ys = {}      # pair -> stage-1 output [128, F] (A|B packed)
                hs = {}      # pair -> list of 6 stage-2 outputs [128, F]
                ot2 = {}     # even pair -> [64, PAIR] out staging (2 pairs)

                def xdma(p):
                    xt[p] = xpool.tile([128, 2, PAIR], mdt, tag="xt",
                                       name=f"xt{p}")
                    nc.sync.dma_start(
                        out=xt[p][:],
                        in_=xr[:, :, p * PAIR : (p + 1) * PAIR])

                def s1(p):
                    # column tiles (128K, 64M): half A accumulates into py
                    # partitions 0-63, half B into 64-127; alternating tile
                    # positions (0,0)/(0,64) overlap on the PE
                    py = ppool.tile([128, F], f32, tag="py", bufs=1)
                    xa = xt[p][:, :, 0:F]
                    xb = xt[p][:, :, F:PAIR]
                    nc.tensor.matmul(py[0:64, :], w1[0], xa[:, 0],
                                     start=True, stop=False)
                    nc.tensor.matmul(py[64:128, :], w1[0], xb[:, 0],
                                     start=True, stop=False)
                    nc.tensor.matmul(py[0:64, :], w1[1], xa[:, 1],
                                     start=False, stop=True)
                    nc.tensor.matmul(py[64:128, :], w1[1], xb[:, 1],
                                     start=False, stop=True)
                    ys[p] = apool.tile([128, F], mdt, tag="ys", bufs=3, name=f"ys{p}")
                    relu_bias("DVE", ys[p][:], py[:], b1)
                    del xt[p]

                def s2(p):
                    # K=64 matmuls on alternating 64-row PE tiles (0,0)/(64,0)
                    # overlap on HW (~1.7x measured): W2T_m is duplicated in
                    # both partition halves of its block; half A contracts
                    # ys[0:64] (= y of tile jA), half B contracts ys[64:128].
                    # Each m's A/B results land in one [128, 2, F] double-bank
                    # psum tile and evict with a single [128, 2F] op.
                    hs[p] = []
                    engs = ("ACT", "DVE", "ACT")
                    for m in range(3):
                        phd = ppool.tile([128, 2, F], f32, tag="ph", bufs=3,
                                         name=f"ph{p}_{m}")
                        for half in range(2):
                            r0 = 64 * half
                            nc.tensor.matmul(phd[:, half, :],
                                             w2[m][r0 : r0 + 64, :],
                                             ys[p][r0 : r0 + 64, :],
                                             start=True, stop=True)
                        hd = apool.tile([128, 2, F], mdt, tag="hs", bufs=9,
                                        name=f"hs{p}_{m}")
                        relu_bias(engs[m], hd[:], phd[:], b2[m])
                        hs[p].extend([hd[:, 0, :], hd[:, 1, :]])
                    del ys[p]

                def s3(p):
                    # column tiles (128K, 32M): half A accumulates into pob
                    # partitions 0-31 (12 real + zero-pad), half B into
                    # 32-63; alternating positions (0,0)/(0,32) overlap
                    pob = ppool.tile([128, F], f32, tag="po", bufs=1)
                    for i in range(6):
                        k, half = i // 2, i % 2
                        c0 = 32 * half
                        nc.tensor.matmul(pob[c0 : c0 + 32, :], w3[k],
                                         hs[p][i],
                                         start=(i < 2), stop=(i >= 4))
                    ot = opool.tile([64, F], mdt, tag="ot")
                    add_bias("DVE", ot[0:44, :], pob[0:44, :], b3)
                    del hs[p]
                    c0 = p * PAIR
                    qa, qb = {"sp": (nc.sync, nc.sync),
                              "split": (nc.scalar, nc.gpsimd),
                              "pool2": (nc.gpsimd, nc.gpsimd)}[OUT_Q]
                    qa.dma_start(out=out[:, c0 : c0 + F],
                                 in_=ot[0:COUT, :])
                    qb.dma_start(out=out[:, c0 + F : c0 + PAIR],
                                 in_=ot[32 : 32 + COUT, :])

                # prologue: pair 0 arrives as two half-tiles so S1(0)'s
                # first matmuls start after ~700 ns of DMA instead of ~1.5 us
                xt[0] = xpool.tile([128, 2, PAIR], mdt, tag="xt", name="xt0")
                nc.sync.dma_start(out=xt[0][:, :, 0:F], in_=xr[:, :, 0:F])
                nc.sync.dma_start(out=xt[0][:, :, F:PAIR],
                                  in_=xr[:, :, F:PAIR])
                xdma(1)
                xdma(2)
                s1(0)

                for p in range(NPAIRS):
                    if p + 3 < NPAIRS:
                        xdma(p + 3)
                    if p + 1 < NPAIRS:
                        s1(p + 1)
                    s2(p)
                    if p >= 2:
                        s3(p - 2)
                s3(NPAIRS - 2)
                s3(NPAIRS - 1)

    nc.finalize()
    _check_matmul_waits(nc)
    return nc


def _check_matmul_waits(nc):
    import concourse.mybir as mybir

    bad = []
    for f in nc.m.functions:
        for blk in f.blocks:
            for inst in blk.instructions:
                if isinstance(inst, mybir.InstMatmult) and inst.sync_info:
                    if len(inst.sync_info.on_wait) > 1:
                        bad.append((inst.name,
                                    [w.ant_name for w in inst.sync_info.on_wait]))
    if bad:
        raise RuntimeError(f"matmuls with >1 sync wait (walrus limit): {bad}")


def _get_nc(mm_dtype: str, repeat: int = 1):
    key = (mm_dtype, repeat, OUT_Q, XT_BUFS)
    if key not in _CACHE:
        _CACHE[key] = _build_bass(mm_dtype, repeat)
    return _CACHE[key]


def _np_mm_dtype(mm_dtype: str):
    if mm_dtype == "bf16":
        import ml_dtypes
        return ml_dtypes.bfloat16
    if mm_dtype == "f16":
        return np.float16
    return np.float32  # f32r streams fp32 bits


def _fold_params(inputs, mm_dtype: str):
    """Fold BN into conv weights; pack into the on-device tile layouts."""
    f = lambda k: np.asarray(inputs[k], np.float32)

    inv1 = f("shared_gamma") / np.sqrt(f("shared_var") + BN_EPS)          # [64]
    W1 = f("shared_w") * inv1[:, None]                                    # [64, 256]
    b1v = f("shared_b") * inv1 + f("shared_beta") - f("shared_mean") * inv1

    inv2 = f("heads_gamma") / np.sqrt(f("heads_var") + BN_EPS)            # [6, 64]
    W2 = (f("heads_w1") * inv2[:, :, None]).reshape(HN * CS, CS)          # [384, 64]
    b2v = (f("heads_b1") * inv2 + f("heads_beta")
           - f("heads_mean") * inv2).reshape(HN * CS)                     # [384]

    hw2, hb2 = f("heads_w2"), f("heads_b2")
    W3 = np.zeros((COUT, HN * CS), np.float32)                            # [12, 384]
    b3v = np.zeros((COUT,), np.float32)
    r = 0
    for h, ch in enumerate(HEAD_CH):
        W3[r : r + ch, h * CS : (h + 1) * CS] = hw2[h, :ch, :]
        b3v[r : r + ch] = hb2[h, :ch]
        r += ch

    # packed stationaries (see module docstring for the layout)
    wpk = np.zeros((128, W_COLS), np.float32)
    for k in range(2):                         # stage-1 k-chunks [128, 64]
        wpk[:, W1_OFF + k * 64 : W1_OFF + (k + 1) * 64] = \
            W1[:, k * 128 : (k + 1) * 128].T
    for m in range(3):                         # stage-2: W2T_m in both halves
        w2m = W2[m * 128 : (m + 1) * 128, :].T                            # [64, 128]
        wpk[0:64, W2_OFF + m * 128 : W2_OFF + (m + 1) * 128] = w2m
        wpk[64:128, W2_OFF + m * 128 : W2_OFF + (m + 1) * 128] = w2m
    for k in range(3):                         # stage-3 k-chunks [128, 32]
        wpk[:, W3_OFF + k * 32 : W3_OFF + k * 32 + COUT] = \
            W3[:, k * 128 : (k + 1) * 128].T

    bpk = np.zeros((128, B_COLS), np.float32)
    bpk[0:64, B1_COL] = b1v
    bpk[64:128, B1_COL] = b1v
    for m in range(3):
        bpk[:, B2_COL + m] = b2v[m * 128 : (m + 1) * 128]
    bpk[0:COUT, B3_COL] = b3v
    bpk[32 : 32 + COUT, B3_COL] = b3v

    wpk = wpk.astype(_np_mm_dtype(mm_dtype))
    return {"wp": wpk, "bp": bpk}


def _run(inputs, mm_dtype=MM_DTYPE, trace=False):
    from concourse.bass_utils import run_bass_kernel_spmd

    nc = _get_nc(mm_dtype)
    shared = _fold_params(inputs, mm_dtype)
    ct = np.asarray(inputs["ct_feat"], np.float32).astype(_np_mm_dtype(mm_dtype))
    in_maps = [
        {"x": np.ascontiguousarray(ct[b]), **shared} for b in range(B)
    ]
    res = run_bass_kernel_spmd(nc, in_maps, core_ids=list(range(NCORES)),
                               trace=trace)
    out = np.stack([np.asarray(res.results[b]["out"], np.float32)
                    for b in range(B)], axis=0)
    return out, res


def kernel(**inputs) -> np.ndarray:
    out, _ = _run(inputs)
    return out


# revision 29
# speedup vs baseline: 2.9848x; 1.2022x over previous
"""CenterFormer bbox head as a fused 3-stage matmul chain on 8 TRN2 cores.

Reference computation (per batch b, per proposal n):
  y = relu(BN(shared_w @ x + shared_b))            # 256 -> 64
  h = relu(BN(heads_w1[h] @ y + heads_b1[h]))      # 64 -> 64, 6 heads
  o = heads_w2[h] @ h + heads_b2[h]                # 64 -> 3 (padded), slice+concat -> 12

Host-side preprocessing folds BN (eval mode) into the conv weights, stacks the
6 head convs into a single [384, 64] matmul, and builds a block-diagonal
[12, 384] final conv that directly emits the channel-concatenated output.

Sharding: data-parallel over batch: core b handles ct_feat[b] ([256, 16384]).

Device kernel design (per core, N=16384 split into 32 tiles of F=512,
processed as 16 pairs):
  - bf16 end-to-end: x cast host-side (halves input DMA, the dominant cost),
    output written bf16 and upcast to f32 on the host.
  - PE-array tiling with measured instruction OVERLAP between disjoint tiles
    (~1.7x): stage 1 uses (128K, 64M) column tiles at positions (0,0)/(0,64)
    -- half A accumulates y(jA) into PSUM partitions 0-63, half B y(jB) into
    64-127; stage 2 uses (64K, 128M) row tiles at (0,0)/(64,0) with W2T_m
    duplicated in both partition halves; stage 3 uses (128K, 32M) column
    tiles at (0,0)/(0,32), so out(jA) lands in partitions 0-11 and out(jB)
    in 32-43 of one bank. Adjacent matmuls always alternate tile positions.
  - PSUM budget: py x1 + ph x6 + po x1 = 8 banks.
  - The PE stream is software-pipelined: iteration p emits
    S1(p+1) | S2(p) | S3(p-2), so a matmul never waits on an eviction that
    was issued less than a full iteration earlier.
  - PSUM evictions (relu+bias / add-bias) are statically assigned to ACT
    and DVE only (GPSIMD cannot read PSUM): E1 ACT, E2 alternating DVE/ACT,
    E3 DVE -- 4 ops per engine per pair.
  - All DMAs trigger from SP; input prefetch runs XT_BUFS pairs deep.
"""

import numpy as np

BN_EPS = 1e-3
HEAD_CH = (3, 2, 1, 3, 2, 1)
B, CIN, N, CS, HN = 8, 256, 16384, 64, 6
COUT = sum(HEAD_CH)  # 12
NCORES = 8

MM_DTYPE = "bf16"

F = 512            # matmul free-dim tile (one fp32 PSUM bank)
PAIR = 2 * F       # two tiles processed per pipeline iteration
NPAIRS = N // PAIR  # 16

# packed stationary layout (columns of the [128, 608] weight tile):
#   cols 0-127:   stage-1: W1T k-chunks, [128, 64] each (shared by A/B halves
#                 via output column tiles)
#   cols 128-511: stage-2: W2T_m [128, 128], duplicated in both partition
#                 halves (row tiles)
#   cols 512-607: stage-3: W3T k-chunks padded to [128, 32] (shared by A/B
#                 via output column tiles)
W1_OFF, W2_OFF, W3_OFF, W_COLS = 0, 128, 512, 608
# bias tile [128, 5] f32: col0 [b1;b1], col1-3 b2 chunks, col4 b3 at rows
# {0-11, 32-43}
B1_COL, B2_COL, B3_COL, B_COLS = 0, 1, 4, 5

_CACHE: dict = {}

# tuning knobs (part of the build cache key)
OUT_Q = "sp"      # "sp": both out-DMAs on SP; "split": one on ACT, one on Pool
XT_BUFS = 6       # input-tile prefetch depth


def _build_bass(mm_dtype: str, repeat: int = 1):
    import concourse.bacc as bacc
    import concourse.mybir as mybir
    from concourse.tile import TileContext

    f32 = mybir.dt.float32
    mdt = {"f32r": mybir.dt.float32r, "bf16": mybir.dt.bfloat16,
           "f16": mybir.dt.float16}[mm_dtype]
    AF = mybir.ActivationFunctionType

    nc = bacc.Bacc()
    x = nc.declare_dram_parameter("x", [CIN, N], mdt, isOutput=False)
    wp = nc.declare_dram_parameter("wp", [128, W_COLS], mdt, isOutput=False)
    bp = nc.declare_dram_parameter("bp", [128, B_COLS], f32, isOutput=False)
    out = nc.declare_dram_parameter("out", [COUT, N], mdt, isOutput=True)

    with TileContext(nc) as tc:
        with (
            tc.tile_pool(name="const", bufs=1) as cpool,
            tc.tile_pool(name="xin", bufs=XT_BUFS) as xpool,
            tc.tile_pool(name="acts", bufs=3) as apool,
            tc.tile_pool(name="outs", bufs=2) as opool,
            tc.tile_pool(name="psum", bufs=2, space="PSUM") as ppool,
        ):
            wt = cpool.tile([128, W_COLS], mdt)
            # stage-1 stationaries first so S1(0) can start early; the
            # stage-2/3 blocks (wpB) follow the first two x tiles so the
            # serialized DMA engines deliver S1's inputs first.
            nc.scalar.dma_start(out=wt[:, 0:W2_OFF], in_=wp[:, 0:W2_OFF])
            bt = cpool.tile([128, B_COLS], f32)
            nc.scalar.dma_start(out=bt[:], in_=bp[:])

            w1 = [wt[:, W1_OFF + k * 64 : W1_OFF + (k + 1) * 64]
                  for k in range(2)]
            w2 = [wt[:, W2_OFF + m * 128 : W2_OFF + (m + 1) * 128]
                  for m in range(3)]
            w3 = [wt[:, W3_OFF + k * 32 : W3_OFF + (k + 1) * 32]
                  for k in range(3)]
            b1 = bt[:, B1_COL : B1_COL + 1]
            b2 = [bt[:, B2_COL + m : B2_COL + m + 1] for m in range(3)]
            b3 = bt[0:44, B3_COL : B3_COL + 1]

            # stage-2/3 stationaries issued separately so the first x tiles
            # win the serialized DMA engines. No warm-up ops: bf16 matmuls
            # lower to Ldweights+Matmult, so bacc can split a matmul's two
            # DMA waits across the pair (_check_matmul_waits verifies).
            nc.scalar.dma_start(out=wt[:, W2_OFF:], in_=wp[:, W2_OFF:])

            xr = x.rearrange("(k p) n -> p k n", p=128)

            import contextlib
            loop_cm = (tc.For_i(0, repeat,
                                hint_engines=(mybir.EngineType.PE,))
                       if repeat > 1 else contextlib.nullcontext())

            def relu_bias(eng, dst, src, bias_ap):
                if eng == "ACT":
                    nc.scalar.activation(dst, src, AF.Relu, bias=bias_ap)
                elif eng == "DVE":
                    nc.vector.tensor_scalar(dst, src, bias_ap, 0.0,
                                            mybir.AluOpType.add,
                                            mybir.AluOpType.max)
                else:
                    nc.gpsimd.tensor_scalar(dst, src, bias_ap, 0.0,
                                            mybir.AluOpType.add,
                                            mybir.AluOpType.max)

            def add_bias(eng, dst, src, bias_ap):
                if eng == "ACT":
                    nc.scalar.activation(dst, src, AF.Identity, bias=bias_ap)
                elif eng == "DVE":
                    nc.vector.tensor_scalar(dst, src, bias_ap, None,
                                            mybir.AluOpType.add)
                else:
                    nc.gpsimd.tensor_scalar(dst, src, bias_ap, None,
                                            mybir.AluOpType.add)

            with loop_cm:
                xt = {}      # even pair -> x tile [128, 2, 2*PAIR] (2 pairs)
                ys = {}      # pair -> stage-1 output [128, F] (A|B packed)
                hs = {}      # pair -> list of 6 stage-2 outputs [128, F]
                ot2 = {}     # even pair -> [64, PAIR] out staging (2 pairs)

                def xdma(p):
                    xt[p] = xpool.tile([128, 2, PAIR], mdt, tag="xt",
                                       name=f"xt{p}")
                    nc.sync.dma_start(
                        out=xt[p][:],
                        in_=xr[:, :, p * PAIR : (p + 1) * PAIR])

                def s1(p):
                    # column tiles (128K, 64M): half A accumulates into py
                    # partitions 0-63, half B into 64-127; alternating tile
                    # positions (0,0)/(0,64) overlap on the PE
                    py = ppool.tile([128, F], f32, tag="py", bufs=1)
                    xa = xt[p][:, :, 0:F]
                    xb = xt[p][:, :, F:PAIR]
                    nc.tensor.matmul(py[0:64, :], w1[0], xa[:, 0],
                                     start=True, stop=False)
                    nc.tensor.matmul(py[64:128, :], w1[0], xb[:, 0],
                                     start=True, stop=False)
                    nc.tensor.matmul(py[0:64, :], w1[1], xa[:, 1],
                                     start=False, stop=True)
                    nc.tensor.matmul(py[64:128, :], w1[1], xb[:, 1],
                                     start=False, stop=True)
                    ys[p] = apool.tile([128, F], mdt, tag="ys", bufs=3, name=f"ys{p}")
                    relu_bias("DVE", ys[p][:], py[:], b1)
                    del xt[p]

                def s2(p):
                    # K=64 matmuls on alternating 64-row PE tiles (0,0)/(64,0)
                    # overlap on HW (~1.7x measured): W2T_m is duplicated in
                    # both partition halves of its block; half A contracts
                    # ys[0:64] (= y of tile jA), half B contracts ys[64:128].
                    # Each m's A/B results land in one [128, 2, F] double-bank
                    # psum tile and evict with a single [128, 2F] op.
                    hs[p] = []
                    engs = ("ACT", "DVE", "ACT")
                    for m in range(3):
                        phd = ppool.tile([128, 2, F], f32, tag="ph", bufs=3,
                                         name=f"ph{p}_{m}")
                        for half in range(2):
                            r0 = 64 * half
                            nc.tensor.matmul(phd[:, half, :],
                                             w2[m][r0 : r0 + 64, :],
                                             ys[p][r0 : r0 + 64, :],
                                             start=True, stop=True)
                        hd = apool.tile([128, 2, F], mdt, tag="hs", bufs=9,
                                        name=f"hs{p}_{m}")
                        relu_bias(engs[m], hd[:], phd[:], b2[m])
                        hs[p].extend([hd[:, 0, :], hd[:, 1, :]])
                    del ys[p]

                def s3(p):
                    # column tiles (128K, 32M): half A accumulates into pob
                    # partitions 0-31 (12 real + zero-pad), half B into
                    # 32-63; alternating positions (0,0)/(0,32) overlap
                    pob = ppool.tile([128, F], f32, tag="po", bufs=1)
                    for i in range(6):
                        k, half = i // 2, i % 2
                        c0 = 32 * half
                        nc.tensor.matmul(pob[c0 : c0 + 32, :], w3[k],
                                         hs[p][i],
                                         start=(i < 2), stop=(i >= 4))
                    ot = opool.tile([64, F], mdt, tag="ot")
                    # last pair's add-bias goes to ACT so the two epilogue
                    # evictions run in parallel instead of queuing on DVE
                    add_bias("ACT" if p == NPAIRS - 1 else "DVE",
                             ot[0:44, :], pob[0:44, :], b3)
                    del hs[p]
                    c0 = p * PAIR
                    qa, qb = {"sp": (nc.sync, nc.sync),
                              "split": (nc.scalar, nc.gpsimd),
                              "pool2": (nc.gpsimd, nc.gpsimd)}[OUT_Q]
                    qa.dma_start(out=out[:, c0 : c0 + F],
                                 in_=ot[0:COUT, :])
                    qb.dma_start(out=out[:, c0 + F : c0 + PAIR],
                                 in_=ot[32 : 32 + COUT, :])

                # prologue: pair 0 arrives as two half-tiles so S1(0)'s
                # first matmuls start after ~700 ns of DMA instead of ~1.5 us
                xt[0] = xpool.tile([128, 2, PAIR], mdt, tag="xt", name="xt0")
                nc.sync.dma_start(out=xt[0][:, :, 0:F], in_=xr[:, :, 0:F])
                nc.sync.dma_start(out=xt[0][:, :, F:PAIR],
                                  in_=xr[:, :, F:PAIR])
                xdma(1)
                xdma(2)
                s1(0)

                for p in range(NPAIRS):
                    if p + 3 < NPAIRS:
                        xdma(p + 3)
                    if p + 1 < NPAIRS:
                        s1(p + 1)
                    s2(p)
                    if p >= 2:
                        s3(p - 2)
                s3(NPAIRS - 2)
                s3(NPAIRS - 1)

    nc.finalize()
    _check_matmul_waits(nc)
    return nc


def _check_matmul_waits(nc):
    import concourse.mybir as mybir

    bad = []
    for f in nc.m.functions:
        for blk in f.blocks:
            for inst in blk.instructions:
                if isinstance(inst, mybir.InstMatmult) and inst.sync_info:
                    if len(inst.sync_info.on_wait) > 1:
                        bad.append((inst.name,
                                    [w.ant_name for w in inst.sync_info.on_wait]))
    if bad:
        raise RuntimeError(f"matmuls with >1 sync wait (walrus limit): {bad}")


def _get_nc(mm_dtype: str, repeat: int = 1):
    key = (mm_dtype, repeat, OUT_Q, XT_BUFS)
    if key not in _CACHE:
        _CACHE[key] = _build_bass(mm_dtype, repeat)
    return _CACHE[key]


def _np_mm_dtype(mm_dtype: str):
    if mm_dtype == "bf16":
        import ml_dtypes
        return ml_dtypes.bfloat16
    if mm_dtype == "f16":
        return np.float16
    return np.float32  # f32r streams fp32 bits


def _fold_params(inputs, mm_dtype: str):
    """Fold BN into conv weights; pack into the on-device tile layouts."""
    f = lambda k: np.asarray(inputs[k], np.float32)

    inv1 = f("shared_gamma") / np.sqrt(f("shared_var") + BN_EPS)          # [64]
    W1 = f("shared_w") * inv1[:, None]                                    # [64, 256]
    b1v = f("shared_b") * inv1 + f("shared_beta") - f("shared_mean") * inv1

    inv2 = f("heads_gamma") / np.sqrt(f("heads_var") + BN_EPS)            # [6, 64]
    W2 = (f("heads_w1") * inv2[:, :, None]).reshape(HN * CS, CS)          # [384, 64]
    b2v = (f("heads_b1") * inv2 + f("heads_beta")
           - f("heads_mean") * inv2).reshape(HN * CS)                     # [384]

    hw2, hb2 = f("heads_w2"), f("heads_b2")
    W3 = np.zeros((COUT, HN * CS), np.float32)                            # [12, 384]
    b3v = np.zeros((COUT,), np.float32)
    r = 0
    for h, ch in enumerate(HEAD_CH):
        W3[r : r + ch, h * CS : (h + 1) * CS] = hw2[h, :ch, :]
        b3v[r : r + ch] = hb2[h, :ch]
        r += ch

    # packed stationaries (see module docstring for the layout)
    wpk = np.zeros((128, W_COLS), np.float32)
    for k in range(2):                         # stage-1 k-chunks [128, 64]
        wpk[:, W1_OFF + k * 64 : W1_OFF + (k + 1) * 64] = \
            W1[:, k * 128 : (k + 1) * 128].T
    for m in range(3):                         # stage-2: W2T_m in both halves
        w2m = W2[m * 128 : (m + 1) * 128, :].T                            # [64, 128]
        wpk[0:64, W2_OFF + m * 128 : W2_OFF + (m + 1) * 128] = w2m
        wpk[64:128, W2_OFF + m * 128 : W2_OFF + (m + 1) * 128] = w2m
    for k in range(3):                         # stage-3 k-chunks [128, 32]
        wpk[:, W3_OFF + k * 32 : W3_OFF + k * 32 + COUT] = \
            W3[:, k * 128 : (k + 1) * 128].T

    bpk = np.zeros((128, B_COLS), np.float32)
    bpk[0:64, B1_COL] = b1v
    bpk[64:128, B1_COL] = b1v
    for m in range(3):
        bpk[:, B2_COL + m] = b2v[m * 128 : (m + 1) * 128]
    bpk[0:COUT, B3_COL] = b3v
    bpk[32 : 32 + COUT, B3_COL] = b3v

    wpk = wpk.astype(_np_mm_dtype(mm_dtype))
    return {"wp": wpk, "bp": bpk}


def _run(inputs, mm_dtype=MM_DTYPE, trace=False):
    from concourse.bass_utils import run_bass_kernel_spmd

    nc = _get_nc(mm_dtype)
    shared = _fold_params(inputs, mm_dtype)
    ct = np.asarray(inputs["ct_feat"], np.float32).astype(_np_mm_dtype(mm_dtype))
    in_maps = [
        {"x": np.ascontiguousarray(ct[b]), **shared} for b in range(B)
    ]
    res = run_bass_kernel_spmd(nc, in_maps, core_ids=list(range(NCORES)),
                               trace=trace)
    out = np.stack([np.asarray(res.results[b]["out"], np.float32)
                    for b in range(B)], axis=0)
    return out, res


def kernel(**inputs) -> np.ndarray:
    out, _ = _run(inputs)
    return out
